# revision 37
# baseline (speedup 1.0000x reference)
"""Trainium2 Bass kernel for per-head Llama GQA attention.

Model: H=16 q heads, HKV=4 kv heads, head_dim=128, L=2048, D=2048, B=1.
Per-head hidden streams and per-head outputs (no cross-head reduction), so
tensor-parallel over heads is embarrassingly parallel: core c owns q heads
{2c, 2c+1} and their kv head c//2.  No collectives.

Schedule (per core), designed so the inbound DMA (29 MB of hidden streams)
and the PE are both near-saturated with no serial phases:
  - stream order: consts -> xk -> xq0 -> xq1 -> xv.  Projection matmuls for
    each stream are emitted per-d-tile so they track the DMA.
  - as soon as K and a Q head are roped, score+exp units for that head are
    interleaved between the projection matmul groups of the later streams:
    by the time xv lands, nearly all 40 score/exp units (both heads) have
    issued, and ACT has drained most of the exp work.
  - causal diagonal blocks are triangle-trimmed: scores/exp/attnV/row-sum
    matmuls for diag j-tile r only cover query columns [128r:512), cutting
    ~37% of the diagonal-band work on PE and ACT.
  - the tail phase is a pure matmul streak: attnV + row-sum accumulation per
    chunk, output projection pipelined one chunk behind, stores overlapped.
  - softmax normalization: ones-matmul row sums -> basis-vector matmuls
    convert the broadcast sums to per-partition columns whose reciprocal is
    folded into the PSUM->SBUF copies of the output projection.

All layouts contract over the partition dim (no on-device transposes):
hidden streams arrive host-transposed (D-major) fp16, weights as W^T tiles,
cos/sin tables fp16 in (HD, L) with 1/sqrt(HD) folded into the q tables,
rotate_half is a 128x128 signed permutation matmul.
"""

import os
import sys

sys.path.insert(0, "/opt/trn_rl_repo")

import numpy as np

import concourse.bass as bass
import concourse.tile as tile
from concourse import bacc, mybir
from concourse.bass_utils import run_bass_kernel_spmd

H, HKV, D, HD, L = 16, 4, 2048, 128, 2048
THETA = 10000.0
NC = 8
HPC = H // NC  # q heads per core (2)
NDT = D // 128  # d-tiles (16)
NLT = L // 128  # l/j tiles (16)
NCH = L // 512  # 512-wide chunks (4)
F16 = mybir.dt.float16
F32 = mybir.dt.float32
EXP = mybir.ActivationFunctionType.Exp
CPY = mybir.ActivationFunctionType.Copy

last_exec_time_ns = None
last_mean_exec_time_ns = None

_programs = {}


def _build_program(causal: bool) -> bass.Bass:
    # Bacc (not plain Bass): its compile() runs the wait-splitting passes
    # (generate_event_semaphores) that walrus requires.
    nc = bacc.Bacc(None, target_bir_lowering=False)

    xq = nc.dram_tensor("xq", [HPC, NDT, 128, L], F16, kind="ExternalInput")
    xk = nc.dram_tensor("xk", [NDT, 128, L], F16, kind="ExternalInput")
    xv = nc.dram_tensor("xv", [NDT, 128, L], F16, kind="ExternalInput")
    wq = nc.dram_tensor("wq", [128, HPC, NDT, 128], F16, kind="ExternalInput")
    wk = nc.dram_tensor("wk", [128, NDT, 128], F16, kind="ExternalInput")
    wv = nc.dram_tensor("wv", [128, NDT, 128], F16, kind="ExternalInput")
    wo = nc.dram_tensor("wo", [128, HPC, D], F16, kind="ExternalInput")
    cosq = nc.dram_tensor("cosq", [128, L], F16, kind="ExternalInput")
    sinq = nc.dram_tensor("sinq", [128, L], F16, kind="ExternalInput")
    cosk = nc.dram_tensor("cosk", [128, L], F16, kind="ExternalInput")
    sink = nc.dram_tensor("sink", [128, L], F16, kind="ExternalInput")
    # misc fp16 constants: [:, :128] rotate-half perm (lhsT), [:, 128:256] ones
    misc = nc.dram_tensor("misc", [128, 256], F16, kind="ExternalInput")
    e1 = nc.dram_tensor("e1", [128, 1], F32, kind="ExternalInput")
    if causal:
        # one lower-triangle 128x128 additive mask tile, shared by every
        # diagonal block (allowed iff j_local <= l_local)
        trimask = nc.dram_tensor("trimask", [128, 128], F32, kind="ExternalInput")
    else:
        maskg = nc.dram_tensor("maskg", [128, NLT // 2, NCH, 1024], F32, kind="ExternalInput")
    out = nc.dram_tensor("out", [HPC, NLT, 128, D], F16, kind="ExternalOutput")

    with tile.TileContext(nc) as tc:
        with (
            tc.tile_pool(name="const", bufs=1) as constp,
            tc.tile_pool(name="xs", bufs=11) as xpool,
            tc.tile_pool(name="persist", bufs=1) as persist,
            tc.tile_pool(name="probs", bufs=1) as probsp,
            tc.tile_pool(name="small", bufs=3) as small,
            tc.tile_pool(name="outs", bufs=3) as outsp,
            tc.tile_pool(name="recs", bufs=9) as recs,
            # PSUM: 8 banks, split by role so the slot rings (which are
            # per-tag) never make a rotating tile wait on a long-held
            # accumulator: "pproj" = 1x(128,1024) held projection
            # accumulator, "sp" = 2x(128,1024) rotating (scores /
            # rotate-half / out-proj / diag extract), "pacc" =
            # 2x(128,512) one-bank accumulators (projection high half
            # during streams, attnT + row sums during attention).
            tc.tile_pool(name="pbig", bufs=1, space="PSUM") as pbig,
            tc.tile_pool(name="pacc", bufs=2, space="PSUM") as paccp,
        ):
            # ---- constants, ordered by first use on the sync DMA queue ----
            misc_sb = constp.tile([128, 256], F16, tag="misc")
            nc.sync.dma_start(out=misc_sb[:], in_=misc[:])
            wk_sb = constp.tile([128, NDT, 128], F16, tag="wk")
            nc.sync.dma_start(out=wk_sb[:], in_=wk[:])
            cosk_sb = constp.tile([128, L], F16, tag="cosk")
            nc.sync.dma_start(out=cosk_sb[:], in_=cosk[:])
            sink_sb = constp.tile([128, L], F16, tag="sink")
            nc.sync.dma_start(out=sink_sb[:], in_=sink[:])
            w_all = constp.tile([128, HPC, NDT, 128], F16, tag="wq")
            nc.sync.dma_start(out=w_all[:], in_=wq[:])
            cosq_sb = constp.tile([128, L], F16, tag="cosq")
            nc.sync.dma_start(out=cosq_sb[:], in_=cosq[:])
            sinq_sb = constp.tile([128, L], F16, tag="sinq")
            nc.sync.dma_start(out=sinq_sb[:], in_=sinq[:])
            e1_sb = constp.tile([128, 1], F32, tag="e1")
            nc.sync.dma_start(out=e1_sb[:], in_=e1[:])
            if causal:
                tri_sb = constp.tile([128, 128], F32, tag="tri")
                nc.sync.dma_start(out=tri_sb[:], in_=trimask[:])
            # wv/wo are not needed until the xv stream / first out-proj;
            # their DMA triggers are issued later on the gpsimd queue so
            # they don't compete with xk/xq0 for HBM bandwidth
            wv_sb = constp.tile([128, NDT, 128], F16, tag="wv")
            wo_sb = constp.tile([128, HPC, D], F16, tag="wo")
            perm = misc_sb[:, 0:128]
            ones = misc_sb[:, 128:256]

            # persistent per-core activations
            krot = persist.tile([128, L], F16, tag="krot")
            v16 = persist.tile([128, L], F16, tag="v16")
            qrot = [
                persist.tile([128, L], F16, tag=f"qrot{i}", name=f"qrot{i}")
                for i in range(HPC)
            ]

            # ---------- score + exp units ----------
            # Each unit issues the score matmuls (1 pbig slot) and the exp
            # for a slice of one head-chunk, leaving the exp'd probs tile
            # held in SBUF for the later attnV/row-sum phase.  Units are
            # interleaved between projection matmul groups of the xq1/xv
            # streams so PE/ACT never idle while DMA streams.
            pe_off = {}  # (i, c, jp) -> [128,1024] f16: j-tiles 2jp|2jp+1
            pe_d1 = {}  # (i, c) -> [128,1024] f16: diag r0 [0:512], r1 [512:896]
            pe_d2 = {}  # (i, c) -> [128,512] f16: diag r2 [0:256], r3 [256:384]

            def unit_off(i, c, jp):
                def emit():
                    qsl = qrot[i][:, c * 512 : (c + 1) * 512]
                    jt0 = 2 * jp
                    sp = pbig.tile([128, 1024], F32, tag="sp", bufs=2)
                    nc.tensor.matmul(
                        sp[:, 0:512], krot[:, jt0 * 128 : (jt0 + 1) * 128], qsl
                    )
                    nc.tensor.matmul(
                        sp[:, 512:1024],
                        krot[:, (jt0 + 1) * 128 : (jt0 + 2) * 128],
                        qsl,
                    )
                    if not causal:
                        mg = small.tile([128, 1024], F32, tag="maskg", bufs=4)
                        nc.vector.dma_start(out=mg[:], in_=maskg[:, jp, c, :])
                        nc.vector.tensor_add(out=sp[:], in0=sp[:], in1=mg[:])
                    pe = probsp.tile(
                        [128, 1024], F16, tag="probs", bufs=25 if causal else 38
                    )
                    nc.scalar.activation(out=pe[:], in_=sp[:], func=EXP)
                    pe_off[(i, c, jp)] = pe

                return emit

            def unit_diag1(i, c):
                def emit():
                    qsl = qrot[i][:, c * 512 : (c + 1) * 512]
                    jb = 4 * c
                    sp = pbig.tile([128, 1024], F32, tag="sp", bufs=2)
                    nc.tensor.matmul(
                        sp[:, 0:512], krot[:, jb * 128 : (jb + 1) * 128], qsl
                    )
                    nc.tensor.matmul(
                        sp[:, 512:896],
                        krot[:, (jb + 1) * 128 : (jb + 2) * 128],
                        qsl[:, 128:512],
                    )
                    nc.vector.tensor_add(
                        out=sp[:, 0:128], in0=sp[:, 0:128], in1=tri_sb[:]
                    )
                    nc.vector.tensor_add(
                        out=sp[:, 512:640], in0=sp[:, 512:640], in1=tri_sb[:]
                    )
                    pe = probsp.tile([128, 1024], F16, tag="probsd", bufs=8)
                    nc.scalar.activation(out=pe[:, 0:896], in_=sp[:, 0:896], func=EXP)
                    pe_d1[(i, c)] = pe

                return emit

            def unit_diag2(i, c):
                def emit():
                    qsl = qrot[i][:, c * 512 : (c + 1) * 512]
                    jb = 4 * c
                    sp = pbig.tile([128, 1024], F32, tag="sp", bufs=2)
                    # two accumulation regions in one PSUM bank: start=True
                    # only on the first (clears the bank's has_written bits);
                    # the second's first write lands on clear bits and
                    # overwrites rather than accumulates.
                    nc.tensor.matmul(
                        sp[:, 0:256],
                        krot[:, (jb + 2) * 128 : (jb + 3) * 128],
                        qsl[:, 256:512],
                        start=True, stop=False, skip_group_check=True,
                    )
                    nc.tensor.matmul(
                        sp[:, 256:384],
                        krot[:, (jb + 3) * 128 : (jb + 4) * 128],
                        qsl[:, 384:512],
                        start=False, stop=True, skip_group_check=True,
                    )
                    nc.vector.tensor_add(
                        out=sp[:, 0:128], in0=sp[:, 0:128], in1=tri_sb[:]
                    )
                    nc.vector.tensor_add(
                        out=sp[:, 256:384], in0=sp[:, 256:384], in1=tri_sb[:]
                    )
                    pe = probsp.tile([128, 512], F16, tag="probsd2", bufs=8)
                    nc.scalar.activation(out=pe[:, 0:384], in_=sp[:, 0:384], func=EXP)
                    pe_d2[(i, c)] = pe

                return emit

            def head_units(i, chunks):
                us = []
                for c in chunks:
                    for jp in range(2 * c if causal else NLT // 2):
                        us.append(((i, c), unit_off(i, c, jp)))
                    if causal:
                        us.append(((i, c), unit_diag1(i, c)))
                        us.append(((i, c), unit_diag2(i, c)))
                return us

            units = []  # filled as q heads become available

            def pop_unit():
                units.pop(0)[1]()

            def drain_until(i, c, extra=4):
                # emit every unit this head-chunk's attnV needs, plus a few
                # lookahead units to keep ACT fed
                while units and (units[0][0] <= (i, c) or extra > 0):
                    if units[0][0] > (i, c):
                        extra -= 1
                    pop_unit()

            # ---------- streamed projections ----------
            def project_rope(x_dram, w_sb_tiles, cos_sb, sin_sb, dst):
                pch_a = pbig.tile([128, 1024], F32, tag="pproj")
                pch_b = [
                    paccp.tile([128, 512], F32, tag="pacc", name=f"pchb{h}")
                    for h in range(2)
                ]
                for dt in range(NDT):
                    xt = xpool.tile([128, L], F16, tag="xtile")
                    nc.gpsimd.dma_start(out=xt[:], in_=x_dram[dt])
                    for c in range(NCH):
                        tgt = (
                            pch_a[:, c * 512 : (c + 1) * 512]
                            if c < 2
                            else pch_b[c - 2][:]
                        )
                        nc.tensor.matmul(
                            tgt,
                            w_sb_tiles[:, dt, :],
                            xt[:, c * 512 : (c + 1) * 512],
                            start=(dt == 0),
                            stop=(dt == NDT - 1),
                        )
                    if units:
                        pop_unit()
                for h in range(2):
                    sl = slice(h * 1024, (h + 1) * 1024)
                    u16 = small.tile([128, 1024], F16, tag="u16", bufs=2)
                    if h == 0:
                        nc.vector.tensor_copy(out=u16[:], in_=pch_a[:])
                    else:
                        nc.vector.tensor_copy(out=u16[:, 0:512], in_=pch_b[0][:])
                        nc.vector.tensor_copy(out=u16[:, 512:1024], in_=pch_b[1][:])
                    rh = pbig.tile([128, 1024], F32, tag="sp", bufs=2)
                    nc.tensor.matmul(rh[:, 0:512], perm, u16[:, 0:512])
                    nc.tensor.matmul(rh[:, 512:1024], perm, u16[:, 512:1024])
                    t0 = small.tile([128, 1024], F16, tag="ropetmp", bufs=2)
                    nc.vector.tensor_mul(out=t0[:], in0=u16[:], in1=cos_sb[:, sl])
                    nc.vector.tensor_mul(out=dst[:, sl], in0=rh[:], in1=sin_sb[:, sl])
                    nc.vector.tensor_add(out=dst[:, sl], in0=dst[:, sl], in1=t0[:])

            # q streams go in column halves: the half's rope completes as
            # soon as its 4.2 MB lands, releasing that half's score units
            # ~9us earlier than a full-stream projection would
            def project_rope_halves(x_dram, w_sb_tiles, cos_sb, sin_sb, dst, i):
                for h in range(2):
                    csl = slice(h * 1024, (h + 1) * 1024)
                    pch = pbig.tile([128, 1024], F32, tag="pproj")
                    for dt in range(NDT):
                        xt = xpool.tile([128, 1024], F16, tag="xtile")
                        nc.gpsimd.dma_start(out=xt[:], in_=x_dram[dt][:, csl])
                        nc.tensor.matmul(
                            pch[:, 0:512],
                            w_sb_tiles[:, dt, :],
                            xt[:, 0:512],
                            start=(dt == 0), stop=(dt == NDT - 1),
                        )
                        nc.tensor.matmul(
                            pch[:, 512:1024],
                            w_sb_tiles[:, dt, :],
                            xt[:, 512:1024],
                            start=(dt == 0), stop=(dt == NDT - 1),
                        )
                        if units:
                            pop_unit()
                    u16 = small.tile([128, 1024], F16, tag="u16", bufs=2)
                    nc.vector.tensor_copy(out=u16[:], in_=pch[:])
                    rh = pbig.tile([128, 1024], F32, tag="sp", bufs=2)
                    nc.tensor.matmul(rh[:, 0:512], perm, u16[:, 0:512])
                    nc.tensor.matmul(rh[:, 512:1024], perm, u16[:, 512:1024])
                    t0 = small.tile([128, 1024], F16, tag="ropetmp", bufs=2)
                    nc.vector.tensor_mul(out=t0[:], in0=u16[:], in1=cos_sb[:, csl])
                    nc.vector.tensor_mul(out=dst[:, csl], in0=rh[:], in1=sin_sb[:, csl])
                    nc.vector.tensor_add(out=dst[:, csl], in0=dst[:, csl], in1=t0[:])
                    units.extend(head_units(i, [2 * h, 2 * h + 1]))

            project_rope(xk, wk_sb, cosk_sb, sink_sb, krot)
            project_rope_halves(xq[0], w_all[:, 0], cosq_sb, sinq_sb, qrot[0], 0)
            nc.gpsimd.dma_start(out=wv_sb[:], in_=wv[:])
            project_rope_halves(xq[1], w_all[:, 1], cosq_sb, sinq_sb, qrot[1], 1)
            nc.gpsimd.dma_start(out=wo_sb[:], in_=wo[:])

            # ---- V projection: v[lt] += xvT[dt][:, lt] .T@ wvT[dt], (L, HD),
            # interleaved with remaining score/exp units ----
            pv_a = pbig.tile([128, 1024], F32, tag="pproj")
            pv_b = [
                paccp.tile([128, 512], F32, tag="pacc", name=f"pvb{s}")
                for s in range(2)
            ]
            for dt in range(NDT):
                xt = xpool.tile([128, L], F16, tag="xtile")
                nc.gpsimd.dma_start(out=xt[:], in_=xv[dt])
                for lt in range(NLT):
                    # 4 accumulation regions share each PSUM bank (see
                    # unit_diag2 for the start/has_written rules)
                    if lt < 8:
                        tgt = pv_a[:, lt * 128 : (lt + 1) * 128]
                    else:
                        tgt = pv_b[(lt - 8) // 4][
                            :, (lt % 4) * 128 : (lt % 4 + 1) * 128
                        ]
                    nc.tensor.matmul(
                        tgt,
                        xt[:, lt * 128 : (lt + 1) * 128],
                        wv_sb[:, dt, :],
                        start=(dt == 0 and lt % 4 == 0),
                        stop=(dt == NDT - 1 and lt % 4 == 3),
                        skip_group_check=True,
                    )
                if units:
                    pop_unit()
            nc.vector.tensor_copy(out=v16[:, 0:1024], in_=pv_a[:])
            nc.vector.tensor_copy(out=v16[:, 1024:1536], in_=pv_b[0][:])
            nc.vector.tensor_copy(out=v16[:, 1536:2048], in_=pv_b[1][:])

            # ---------- attnV + row sums per head-chunk ----------
            def attn_phase(i, c):
                psums = paccp.tile([128, 512], F32, tag="pacc")
                pattn = paccp.tile([128, 512], F32, tag="pacc")
                npair = 2 * c if causal else NLT // 2
                for jp in range(npair):
                    jt0 = 2 * jp
                    pe = pe_off.pop((i, c, jp))
                    last = (not causal) and jp == npair - 1
                    nc.tensor.matmul(
                        pattn[:],
                        v16[:, jt0 * 128 : (jt0 + 1) * 128],
                        pe[:, 0:512],
                        start=(jp == 0), stop=False, skip_group_check=True,
                    )
                    nc.tensor.matmul(
                        pattn[:],
                        v16[:, (jt0 + 1) * 128 : (jt0 + 2) * 128],
                        pe[:, 512:1024],
                        start=False, stop=last, skip_group_check=True,
                    )
                    # row sums only need one output partition: a [128,1]
                    # ones column as lhsT skips the 128-col weight load
                    nc.tensor.matmul(
                        psums[0:1, :], ones[:, 0:1], pe[:, 0:512],
                        start=(jp == 0), stop=False, skip_group_check=True,
                    )
                    nc.tensor.matmul(
                        psums[0:1, :], ones[:, 0:1], pe[:, 512:1024],
                        start=False, stop=last, skip_group_check=True,
                    )
                    pop_op()
                if causal:
                    jb = 4 * c
                    p1 = pe_d1.pop((i, c))
                    p2 = pe_d2.pop((i, c))
                    segs = [
                        (p1[:, 0:512], 0),
                        (p1[:, 512:896], 128),
                        (p2[:, 0:256], 256),
                        (p2[:, 256:384], 384),
                    ]
                    for r, (pes, col) in enumerate(segs):
                        vsl = v16[:, (jb + r) * 128 : (jb + r + 1) * 128]
                        nc.tensor.matmul(
                            pattn[:, col:512], vsl, pes,
                            start=(c == 0 and r == 0), stop=(r == 3),
                            skip_group_check=True,
                        )
                    pop_op()
                    for r, (pes, col) in enumerate(segs):
                        nc.tensor.matmul(
                            psums[0:1, col:512], ones[:, 0:1], pes,
                            start=(c == 0 and r == 0), stop=(r == 3),
                            skip_group_check=True,
                        )
                    pop_op()
                # drain the accumulators: row sums + unnormalized attnT
                sums32 = small.tile([1, 512], F32, tag="sums32")
                nc.vector.tensor_copy(out=sums32[:], in_=psums[0:1, :])
                attn16 = small.tile([128, 512], F16, tag="attn16")
                nc.vector.tensor_copy(out=attn16[:], in_=pattn[:])
                # broadcast row sums -> per-partition columns, written into
                # the (now drained) psums bank's last columns, and take the
                # reciprocal eagerly so it's long ready when the out-proj
                # units pop
                for ls in range(4):
                    nc.tensor.matmul(
                        psums[:, 508 + ls : 509 + ls],
                        sums32[0:1, ls * 128 : (ls + 1) * 128],
                        e1_sb[0:1, :],
                    )
                recip = recs.tile([128, 4], F32, tag="recip")
                nc.vector.reciprocal(out=recip[:], in_=psums[:, 508:512])
                return recip, attn16

            op_queue = []  # per-l-tile out-proj emitters, interleaved into
            # the next chunk's attnV stream so the drain engines keep pace

            def outproj_units(i, c, recip, attn16):
                def ltile(ls):
                    def emit():
                        lt = 4 * c + ls
                        a_sl = attn16[:, ls * 128 : (ls + 1) * 128]
                        r_sl = recip[:, ls : ls + 1]
                        ost = outsp.tile([128, D], F16, tag="ost")
                        for dp in range(2):
                            # cycle po through sp(x2) + pproj(x1) slots: a
                            # 3-deep ring so PE doesn't wait on the drains
                            k = 2 * ls + dp
                            if k % 3 == 2:
                                po = pbig.tile([128, 1024], F32, tag="pproj")
                            else:
                                po = pbig.tile([128, 1024], F32, tag="sp", bufs=2)
                            nc.tensor.matmul(
                                po[:, 0:512],
                                a_sl,
                                wo_sb[:, i, dp * 1024 : dp * 1024 + 512],
                            )
                            nc.tensor.matmul(
                                po[:, 512:1024],
                                a_sl,
                                wo_sb[:, i, dp * 1024 + 512 : dp * 1024 + 1024],
                            )
                            osl = slice(dp * 1024, (dp + 1) * 1024)
                            # one full-width scaled drain per po tile,
                            # alternating engines: a single 1024-wide op
                            # amortizes the per-instruction overhead
                            if dp == 0:
                                nc.vector.tensor_scalar_mul(
                                    out=ost[:, osl], in0=po[:], scalar1=r_sl
                                )
                            else:
                                nc.scalar.activation(
                                    out=ost[:, osl], in_=po[:],
                                    func=CPY, scale=r_sl,
                                )
                            # store each half as soon as its drain lands so
                            # the final store isn't serialized behind both
                            nc.scalar.dma_start(
                                out=out[i, lt][:, osl], in_=ost[:, osl]
                            )

                    return emit

                return [ltile(ls) for ls in range(4)]

            def pop_op():
                if op_queue:
                    op_queue.pop(0)()

            # head-1 chunks rotated so the serial tail (last attn_phase +
            # its out-proj) lands on the cheapest chunk (c=0, diag only)
            order = [(0, c) for c in range(NCH)] + [
                (1, 1), (1, 2), (1, 3), (1, 0)
            ]
            for i, c in order:
                drain_until(i, c)
                recip, attn16 = attn_phase(i, c)
                op_queue.extend(outproj_units(i, c, recip, attn16))
            while op_queue:
                pop_op()
    nc.compile()
    return nc


def _get_program(causal: bool) -> bass.Bass:
    if causal not in _programs:
        _programs[causal] = _build_program(causal)
    return _programs[causal]


def _rope_tables(position_ids: np.ndarray):
    pos = position_ids.reshape(-1).astype(np.float32)  # (L,)
    inv_freq = (
        1.0 / (THETA ** (np.arange(0, HD, 2, dtype=np.float32) / HD))
    ).astype(np.float32)
    freqs = pos[:, None] * inv_freq[None, :]  # (L, HD/2)
    emb = np.concatenate([freqs, freqs], axis=1)  # (L, HD)
    cos = np.cos(emb).T.astype(np.float32).copy()  # (HD, L)
    sin = np.sin(emb).T.astype(np.float32).copy()
    return cos, sin


def _xt_tiles(x):  # (L, D) fp32 -> (NDT, 128, L) fp16 transposed tiles
    return np.ascontiguousarray(x.T.astype(np.float16).reshape(NDT, 128, L))


def kernel(
    q_hidden, k_hidden, v_hidden, wq, wk, wv, wo, attention_mask, position_ids
):
    global last_exec_time_ns, last_mean_exec_time_ns
    q_hidden = np.asarray(q_hidden)
    k_hidden = np.asarray(k_hidden)
    v_hidden = np.asarray(v_hidden)
    wq = np.asarray(wq, dtype=np.float32)
    wk = np.asarray(wk, dtype=np.float32)
    wv = np.asarray(wv, dtype=np.float32)
    wo = np.asarray(wo, dtype=np.float32)
    attention_mask = np.asarray(attention_mask, dtype=np.float32)
    position_ids = np.asarray(position_ids)

    mask2d = attention_mask.reshape(L, L)
    causal_ref = np.where(
        np.tril(np.ones((L, L), dtype=bool)), np.float32(0.0), np.float32(-1e9)
    )
    causal = bool(np.array_equal(mask2d, causal_ref))

    cos, sin = _rope_tables(position_ids)
    scale = np.float32(1.0 / np.sqrt(HD))
    cosq_h = (cos * scale).astype(np.float16)
    sinq_h = (sin * scale).astype(np.float16)
    cosk_h = cos.astype(np.float16)
    sink_h = sin.astype(np.float16)

    jj = np.arange(128, dtype=np.int32)[:, None]
    ll = np.arange(128, dtype=np.int32)[None, :]
    trimask_h = np.where(jj <= ll, np.float32(0.0), np.float32(-1e9)).astype(
        np.float32
    )

    misc_h = np.zeros((128, 256), dtype=np.float16)
    # rotate-half: rh = P @ q with P[i, i+64] = -1 (i<64), P[i, i-64] = +1;
    # stored as lhsT = P^T
    for a in range(64):
        misc_h[a, a + 64] = np.float16(1.0)  # P^T[a, a+64] = P[a+64, a] = +1
        misc_h[a + 64, a] = np.float16(-1.0)  # P^T[a+64, a] = P[a, a+64] = -1
    misc_h[:, 128:256] = np.float16(1.0)  # ones block
    e1_h = np.zeros((128, 1), dtype=np.float32)
    e1_h[0, 0] = 1.0

    wq_r = wq.reshape(H, HD, D)
    wk_r = wk.reshape(HKV, HD, D)
    wv_r = wv.reshape(HKV, HD, D)
    wo_r = wo.reshape(D, H, HD)

    if not causal:
        # (128, NLT//2, NCH, 1024): pair jp holds j-tiles 2jp | 2jp+1
        mt = mask2d.T.reshape(NLT, 128, NCH, 512)
        maskg_h = np.ascontiguousarray(
            np.concatenate([mt[0::2], mt[1::2]], axis=3).transpose(1, 0, 2, 3)
        ).astype(np.float32)

    in_maps = []
    for core in range(NC):
        heads = [HPC * core + i for i in range(HPC)]
        g = heads[0] // (H // HKV)
        # weights: lhsT layout W^T tiles, partition-major
        wq_t = np.stack(
            [
                wq_r[n].T.astype(np.float16).reshape(NDT, 128, HD)
                for n in heads
            ],
            axis=0,
        )  # (HPC, NDT, 128p, 128m)
        wq_t = np.ascontiguousarray(wq_t.transpose(2, 0, 1, 3))  # (128, HPC, NDT, 128)
        wk_t = wk_r[g].T.astype(np.float16).reshape(NDT, 128, HD)
        wk_t = np.ascontiguousarray(wk_t.transpose(1, 0, 2))  # (128, NDT, 128)
        wv_t = wv_r[g].T.astype(np.float16).reshape(NDT, 128, HD)
        wv_t = np.ascontiguousarray(wv_t.transpose(1, 0, 2))
        wo_t = np.stack(
            [wo_r[:, n, :].T.astype(np.float16) for n in heads], axis=0
        )  # (HPC, 128, D)
        wo_t = np.ascontiguousarray(wo_t.transpose(1, 0, 2))  # (128, HPC, D)

        m = {
            "xq": np.stack([_xt_tiles(q_hidden[n, 0]) for n in heads], axis=0),
            "xk": _xt_tiles(k_hidden[g, 0]),
            "xv": _xt_tiles(v_hidden[g, 0]),
            "wq": wq_t,
            "wk": wk_t,
            "wv": wv_t,
            "wo": wo_t,
            "cosq": cosq_h,
            "sinq": sinq_h,
            "cosk": cosk_h,
            "sink": sink_h,
            "misc": misc_h,
            "e1": e1_h,
        }
        if causal:
            m["trimask"] = trimask_h
        else:
            m["maskg"] = maskg_h
        in_maps.append(m)

    nc = _get_program(causal)
    trace_env = os.environ.get("KERNEL_TRACE", "0")
    kwargs = {}
    if trace_env != "0":
        kwargs["trace"] = True
        if trace_env == "8":
            kwargs["trace_cores"] = list(range(NC))
    res = run_bass_kernel_spmd(nc, in_maps, core_ids=list(range(NC)), **kwargs)
    last_exec_time_ns = res.exec_time_ns
    last_mean_exec_time_ns = res.mean_exec_time_ns
    globals()["last_results"] = res.results
    globals()["last_in_maps"] = in_maps
    globals()["last_res"] = res

    out = np.empty((H, 1, L, D), dtype=np.float32)
    for core in range(NC):
        o = res.results[core]["out"]  # (HPC, NLT, 128, D) fp16
        for i in range(HPC):
            out[HPC * core + i, 0] = o[i].reshape(L, D).astype(np.float32)
    return out


# revision 38
# speedup vs baseline: 1.0565x; 1.0565x over previous
"""Trainium2 Bass kernel for per-head Llama GQA attention.

Model: H=16 q heads, HKV=4 kv heads, head_dim=128, L=2048, D=2048, B=1.
Per-head hidden streams and per-head outputs (no cross-head reduction), so
tensor-parallel over heads is embarrassingly parallel: core c owns q heads
{2c, 2c+1} and their kv head c//2.  No collectives.

Schedule (per core), designed so the inbound DMA (29 MB of hidden streams)
and the PE are both near-saturated with no serial phases:
  - stream order: consts -> xk -> xq0 -> xq1 -> xv.  Projection matmuls for
    each stream are emitted per-d-tile so they track the DMA.
  - as soon as K and a Q head are roped, score+exp units for that head are
    interleaved between the projection matmul groups of the later streams:
    by the time xv lands, nearly all 40 score/exp units (both heads) have
    issued, and ACT has drained most of the exp work.
  - causal diagonal blocks are triangle-trimmed: scores/exp/attnV/row-sum
    matmuls for diag j-tile r only cover query columns [128r:512), cutting
    ~37% of the diagonal-band work on PE and ACT.
  - the tail phase is a pure matmul streak: attnV + row-sum accumulation per
    chunk, output projection pipelined one chunk behind, stores overlapped.
  - softmax normalization: ones-matmul row sums -> basis-vector matmuls
    convert the broadcast sums to per-partition columns whose reciprocal is
    folded into the PSUM->SBUF copies of the output projection.

All layouts contract over the partition dim (no on-device transposes):
hidden streams arrive host-transposed (D-major) fp16, weights as W^T tiles,
cos/sin tables fp16 in (HD, L) with 1/sqrt(HD) folded into the q tables,
rotate_half is a 128x128 signed permutation matmul.
"""

import os
import sys

sys.path.insert(0, "/opt/trn_rl_repo")

import numpy as np

import concourse.bass as bass
import concourse.tile as tile
from concourse import bacc, mybir
from concourse.bass_utils import run_bass_kernel_spmd

H, HKV, D, HD, L = 16, 4, 2048, 128, 2048
THETA = 10000.0
NC = 8
HPC = H // NC  # q heads per core (2)
NDT = D // 128  # d-tiles (16)
NLT = L // 128  # l/j tiles (16)
NCH = L // 512  # 512-wide chunks (4)
F16 = mybir.dt.float16
F32 = mybir.dt.float32
EXP = mybir.ActivationFunctionType.Exp
CPY = mybir.ActivationFunctionType.Copy

last_exec_time_ns = None
last_mean_exec_time_ns = None

_programs = {}


def _build_program(causal: bool) -> bass.Bass:
    # Bacc (not plain Bass): its compile() runs the wait-splitting passes
    # (generate_event_semaphores) that walrus requires.
    nc = bacc.Bacc(None, target_bir_lowering=False)

    xq = nc.dram_tensor("xq", [HPC, NDT, 128, L], F16, kind="ExternalInput")
    xk = nc.dram_tensor("xk", [NDT, 128, L], F16, kind="ExternalInput")
    xv = nc.dram_tensor("xv", [NDT, 128, L], F16, kind="ExternalInput")
    wq = nc.dram_tensor("wq", [128, HPC, NDT, 128], F16, kind="ExternalInput")
    wk = nc.dram_tensor("wk", [128, NDT, 128], F16, kind="ExternalInput")
    wv = nc.dram_tensor("wv", [128, NDT, 128], F16, kind="ExternalInput")
    wo = nc.dram_tensor("wo", [128, HPC, D], F16, kind="ExternalInput")
    cosq = nc.dram_tensor("cosq", [128, L], F16, kind="ExternalInput")
    sinq = nc.dram_tensor("sinq", [128, L], F16, kind="ExternalInput")
    cosk = nc.dram_tensor("cosk", [128, L], F16, kind="ExternalInput")
    sink = nc.dram_tensor("sink", [128, L], F16, kind="ExternalInput")
    # misc fp16 constants: [:, :128] rotate-half perm (lhsT), [:, 128:256] ones
    misc = nc.dram_tensor("misc", [128, 256], F16, kind="ExternalInput")
    e1 = nc.dram_tensor("e1", [128, 1], F32, kind="ExternalInput")
    if causal:
        # one lower-triangle 128x128 additive mask tile, shared by every
        # diagonal block (allowed iff j_local <= l_local)
        trimask = nc.dram_tensor("trimask", [128, 128], F32, kind="ExternalInput")
    else:
        maskg = nc.dram_tensor("maskg", [128, NLT // 2, NCH, 1024], F32, kind="ExternalInput")
    out = nc.dram_tensor("out", [HPC, NLT, 128, D], F16, kind="ExternalOutput")

    with tile.TileContext(nc) as tc:
        with (
            tc.tile_pool(name="const", bufs=1) as constp,
            tc.tile_pool(name="xs", bufs=11) as xpool,
            tc.tile_pool(name="persist", bufs=1) as persist,
            tc.tile_pool(name="probs", bufs=1) as probsp,
            tc.tile_pool(name="small", bufs=3) as small,
            tc.tile_pool(name="outs", bufs=3) as outsp,
            tc.tile_pool(name="recs", bufs=9) as recs,
            # PSUM: 8 banks, split by role so the slot rings (which are
            # per-tag) never make a rotating tile wait on a long-held
            # accumulator: "pproj" = 1x(128,1024) held projection
            # accumulator, "sp" = 2x(128,1024) rotating (scores /
            # rotate-half / out-proj / diag extract), "pacc" =
            # 2x(128,512) one-bank accumulators (projection high half
            # during streams, attnT + row sums during attention).
            tc.tile_pool(name="pbig", bufs=1, space="PSUM") as pbig,
            tc.tile_pool(name="pacc", bufs=2, space="PSUM") as paccp,
        ):
            # ---- constants, ordered by first use on the sync DMA queue ----
            misc_sb = constp.tile([128, 256], F16, tag="misc")
            nc.sync.dma_start(out=misc_sb[:], in_=misc[:])
            wk_sb = constp.tile([128, NDT, 128], F16, tag="wk")
            nc.sync.dma_start(out=wk_sb[:], in_=wk[:])
            cosk_sb = constp.tile([128, L], F16, tag="cosk")
            nc.sync.dma_start(out=cosk_sb[:], in_=cosk[:])
            sink_sb = constp.tile([128, L], F16, tag="sink")
            nc.sync.dma_start(out=sink_sb[:], in_=sink[:])
            w_all = constp.tile([128, HPC, NDT, 128], F16, tag="wq")
            nc.sync.dma_start(out=w_all[:], in_=wq[:])
            cosq_sb = constp.tile([128, L], F16, tag="cosq")
            nc.sync.dma_start(out=cosq_sb[:], in_=cosq[:])
            sinq_sb = constp.tile([128, L], F16, tag="sinq")
            nc.sync.dma_start(out=sinq_sb[:], in_=sinq[:])
            e1_sb = constp.tile([128, 1], F32, tag="e1")
            nc.sync.dma_start(out=e1_sb[:], in_=e1[:])
            if causal:
                tri_sb = constp.tile([128, 128], F32, tag="tri")
                nc.sync.dma_start(out=tri_sb[:], in_=trimask[:])
            # wv/wo are not needed until the xv stream / first out-proj;
            # their DMA triggers are issued later on the gpsimd queue so
            # they don't compete with xk/xq0 for HBM bandwidth
            wv_sb = constp.tile([128, NDT, 128], F16, tag="wv")
            wo_sb = constp.tile([128, HPC, D], F16, tag="wo")
            perm = misc_sb[:, 0:128]
            ones = misc_sb[:, 128:256]

            # persistent per-core activations
            krot = persist.tile([128, L], F16, tag="krot")
            v16 = persist.tile([128, L], F16, tag="v16")
            qrot = [
                persist.tile([128, L], F16, tag=f"qrot{i}", name=f"qrot{i}")
                for i in range(HPC)
            ]

            # ---------- score + exp units ----------
            # Each unit issues the score matmuls (1 pbig slot) and the exp
            # for a slice of one head-chunk, leaving the exp'd probs tile
            # held in SBUF for the later attnV/row-sum phase.  Units are
            # interleaved between projection matmul groups of the xq1/xv
            # streams so PE/ACT never idle while DMA streams.
            pe_off = {}  # (i, c, jp) -> [128,1024] f16: j-tiles 2jp|2jp+1
            pe_d1 = {}  # (i, c) -> [128,1024] f16: diag r0 [0:512], r1 [512:896]
            pe_d2 = {}  # (i, c) -> [128,512] f16: diag r2 [0:256], r3 [256:384]

            def unit_off(i, c, jp):
                def emit():
                    qsl = qrot[i][:, c * 512 : (c + 1) * 512]
                    jt0 = 2 * jp
                    sp = pbig.tile([128, 1024], F32, tag="sp", bufs=2)
                    nc.tensor.matmul(
                        sp[:, 0:512], krot[:, jt0 * 128 : (jt0 + 1) * 128], qsl
                    )
                    nc.tensor.matmul(
                        sp[:, 512:1024],
                        krot[:, (jt0 + 1) * 128 : (jt0 + 2) * 128],
                        qsl,
                    )
                    if not causal:
                        mg = small.tile([128, 1024], F32, tag="maskg", bufs=4)
                        nc.vector.dma_start(out=mg[:], in_=maskg[:, jp, c, :])
                        nc.vector.tensor_add(out=sp[:], in0=sp[:], in1=mg[:])
                    pe = probsp.tile(
                        [128, 1024], F16, tag="probs", bufs=25 if causal else 38
                    )
                    nc.scalar.activation(out=pe[:], in_=sp[:], func=EXP)
                    pe_off[(i, c, jp)] = pe

                return emit

            def unit_diag1(i, c):
                def emit():
                    qsl = qrot[i][:, c * 512 : (c + 1) * 512]
                    jb = 4 * c
                    sp = pbig.tile([128, 1024], F32, tag="sp", bufs=2)
                    nc.tensor.matmul(
                        sp[:, 0:512], krot[:, jb * 128 : (jb + 1) * 128], qsl
                    )
                    nc.tensor.matmul(
                        sp[:, 512:896],
                        krot[:, (jb + 1) * 128 : (jb + 2) * 128],
                        qsl[:, 128:512],
                    )
                    nc.vector.tensor_add(
                        out=sp[:, 0:128], in0=sp[:, 0:128], in1=tri_sb[:]
                    )
                    nc.vector.tensor_add(
                        out=sp[:, 512:640], in0=sp[:, 512:640], in1=tri_sb[:]
                    )
                    pe = probsp.tile([128, 1024], F16, tag="probsd", bufs=8)
                    nc.scalar.activation(out=pe[:, 0:896], in_=sp[:, 0:896], func=EXP)
                    pe_d1[(i, c)] = pe

                return emit

            def unit_diag2(i, c):
                def emit():
                    qsl = qrot[i][:, c * 512 : (c + 1) * 512]
                    jb = 4 * c
                    sp = pbig.tile([128, 1024], F32, tag="sp", bufs=2)
                    # two accumulation regions in one PSUM bank: start=True
                    # only on the first (clears the bank's has_written bits);
                    # the second's first write lands on clear bits and
                    # overwrites rather than accumulates.
                    nc.tensor.matmul(
                        sp[:, 0:256],
                        krot[:, (jb + 2) * 128 : (jb + 3) * 128],
                        qsl[:, 256:512],
                        start=True, stop=False, skip_group_check=True,
                    )
                    nc.tensor.matmul(
                        sp[:, 256:384],
                        krot[:, (jb + 3) * 128 : (jb + 4) * 128],
                        qsl[:, 384:512],
                        start=False, stop=True, skip_group_check=True,
                    )
                    nc.vector.tensor_add(
                        out=sp[:, 0:128], in0=sp[:, 0:128], in1=tri_sb[:]
                    )
                    nc.vector.tensor_add(
                        out=sp[:, 256:384], in0=sp[:, 256:384], in1=tri_sb[:]
                    )
                    pe = probsp.tile([128, 512], F16, tag="probsd2", bufs=8)
                    nc.scalar.activation(out=pe[:, 0:384], in_=sp[:, 0:384], func=EXP)
                    pe_d2[(i, c)] = pe

                return emit

            def head_units(i, chunks):
                us = []
                for c in chunks:
                    for jp in range(2 * c if causal else NLT // 2):
                        us.append(((i, c), unit_off(i, c, jp)))
                    if causal:
                        us.append(((i, c), unit_diag1(i, c)))
                        us.append(((i, c), unit_diag2(i, c)))
                return us

            units = []  # filled as q heads become available

            def pop_unit():
                units.pop(0)[1]()

            def drain_until(i, c, extra=4):
                # emit every unit this head-chunk's attnV needs, plus a few
                # lookahead units to keep ACT fed
                while units and (units[0][0] <= (i, c) or extra > 0):
                    if units[0][0] > (i, c):
                        extra -= 1
                    pop_unit()

            # ---------- streamed projections ----------
            def project_rope(x_dram, w_sb_tiles, cos_sb, sin_sb, dst):
                pch_a = pbig.tile([128, 1024], F32, tag="pproj")
                pch_b = [
                    paccp.tile([128, 512], F32, tag="pacc", name=f"pchb{h}")
                    for h in range(2)
                ]
                for dt in range(NDT):
                    xt = xpool.tile([128, L], F16, tag="xtile")
                    nc.gpsimd.dma_start(out=xt[:], in_=x_dram[dt])
                    for c in range(NCH):
                        tgt = (
                            pch_a[:, c * 512 : (c + 1) * 512]
                            if c < 2
                            else pch_b[c - 2][:]
                        )
                        nc.tensor.matmul(
                            tgt,
                            w_sb_tiles[:, dt, :],
                            xt[:, c * 512 : (c + 1) * 512],
                            start=(dt == 0),
                            stop=(dt == NDT - 1),
                        )
                    if units:
                        pop_unit()
                for h in range(2):
                    sl = slice(h * 1024, (h + 1) * 1024)
                    u16 = small.tile([128, 1024], F16, tag="u16", bufs=2)
                    if h == 0:
                        nc.vector.tensor_copy(out=u16[:], in_=pch_a[:])
                    else:
                        nc.vector.tensor_copy(out=u16[:, 0:512], in_=pch_b[0][:])
                        nc.vector.tensor_copy(out=u16[:, 512:1024], in_=pch_b[1][:])
                    rh = pbig.tile([128, 1024], F32, tag="sp", bufs=2)
                    nc.tensor.matmul(rh[:, 0:512], perm, u16[:, 0:512])
                    nc.tensor.matmul(rh[:, 512:1024], perm, u16[:, 512:1024])
                    t0 = small.tile([128, 1024], F16, tag="ropetmp", bufs=2)
                    nc.vector.tensor_mul(out=t0[:], in0=u16[:], in1=cos_sb[:, sl])
                    nc.vector.tensor_mul(out=dst[:, sl], in0=rh[:], in1=sin_sb[:, sl])
                    nc.vector.tensor_add(out=dst[:, sl], in0=dst[:, sl], in1=t0[:])

            # q streams go in column halves: the half's rope completes as
            # soon as its 4.2 MB lands, releasing that half's score units
            # ~9us earlier than a full-stream projection would
            def project_rope_halves(x_dram, w_sb_tiles, cos_sb, sin_sb, dst, i):
                for h in range(2):
                    csl = slice(h * 1024, (h + 1) * 1024)
                    pch = pbig.tile([128, 1024], F32, tag="pproj")
                    for dt in range(NDT):
                        xt = xpool.tile([128, 1024], F16, tag="xtile")
                        nc.gpsimd.dma_start(out=xt[:], in_=x_dram[dt][:, csl])
                        nc.tensor.matmul(
                            pch[:, 0:512],
                            w_sb_tiles[:, dt, :],
                            xt[:, 0:512],
                            start=(dt == 0), stop=(dt == NDT - 1),
                        )
                        nc.tensor.matmul(
                            pch[:, 512:1024],
                            w_sb_tiles[:, dt, :],
                            xt[:, 512:1024],
                            start=(dt == 0), stop=(dt == NDT - 1),
                        )
                        if units:
                            pop_unit()
                    u16 = small.tile([128, 1024], F16, tag="u16", bufs=2)
                    nc.vector.tensor_copy(out=u16[:], in_=pch[:])
                    rh = pbig.tile([128, 1024], F32, tag="sp", bufs=2)
                    nc.tensor.matmul(rh[:, 0:512], perm, u16[:, 0:512])
                    nc.tensor.matmul(rh[:, 512:1024], perm, u16[:, 512:1024])
                    t0 = small.tile([128, 1024], F16, tag="ropetmp", bufs=2)
                    nc.vector.tensor_mul(out=t0[:], in0=u16[:], in1=cos_sb[:, csl])
                    nc.vector.tensor_mul(out=dst[:, csl], in0=rh[:], in1=sin_sb[:, csl])
                    nc.vector.tensor_add(out=dst[:, csl], in0=dst[:, csl], in1=t0[:])
                    units.extend(head_units(i, [2 * h, 2 * h + 1]))

            project_rope(xk, wk_sb, cosk_sb, sink_sb, krot)
            project_rope_halves(xq[0], w_all[:, 0], cosq_sb, sinq_sb, qrot[0], 0)
            nc.gpsimd.dma_start(out=wv_sb[:], in_=wv[:])
            project_rope_halves(xq[1], w_all[:, 1], cosq_sb, sinq_sb, qrot[1], 1)
            nc.gpsimd.dma_start(out=wo_sb[:], in_=wo[:])

            # ---- V projection: v[lt] += xvT[dt][:, lt] .T@ wvT[dt], (L, HD),
            # interleaved with remaining score/exp units ----
            pv_a = pbig.tile([128, 1024], F32, tag="pproj")
            pv_b = [
                paccp.tile([128, 512], F32, tag="pacc", name=f"pvb{s}")
                for s in range(2)
            ]
            for dt in range(NDT):
                xt = xpool.tile([128, L], F16, tag="xtile")
                nc.gpsimd.dma_start(out=xt[:], in_=xv[dt])
                for lt in range(NLT):
                    # 4 accumulation regions share each PSUM bank (see
                    # unit_diag2 for the start/has_written rules)
                    if lt < 8:
                        tgt = pv_a[:, lt * 128 : (lt + 1) * 128]
                    else:
                        tgt = pv_b[(lt - 8) // 4][
                            :, (lt % 4) * 128 : (lt % 4 + 1) * 128
                        ]
                    nc.tensor.matmul(
                        tgt,
                        xt[:, lt * 128 : (lt + 1) * 128],
                        wv_sb[:, dt, :],
                        start=(dt == 0 and lt % 4 == 0),
                        stop=(dt == NDT - 1 and lt % 4 == 3),
                        skip_group_check=True,
                    )
                if units:
                    pop_unit()
            nc.vector.tensor_copy(out=v16[:, 0:1024], in_=pv_a[:])
            nc.vector.tensor_copy(out=v16[:, 1024:1536], in_=pv_b[0][:])
            nc.vector.tensor_copy(out=v16[:, 1536:2048], in_=pv_b[1][:])

            # ---------- attnV + row sums per head-chunk ----------
            def attn_phase(i, c):
                psums = paccp.tile([128, 512], F32, tag="pacc")
                pattn = paccp.tile([128, 512], F32, tag="pacc")
                npair = 2 * c if causal else NLT // 2
                for jp in range(npair):
                    jt0 = 2 * jp
                    pe = pe_off.pop((i, c, jp))
                    last = (not causal) and jp == npair - 1
                    nc.tensor.matmul(
                        pattn[:],
                        v16[:, jt0 * 128 : (jt0 + 1) * 128],
                        pe[:, 0:512],
                        start=(jp == 0), stop=False, skip_group_check=True,
                    )
                    nc.tensor.matmul(
                        pattn[:],
                        v16[:, (jt0 + 1) * 128 : (jt0 + 2) * 128],
                        pe[:, 512:1024],
                        start=False, stop=last, skip_group_check=True,
                    )
                    # row sums only need one output partition: a [128,1]
                    # ones column as lhsT skips the 128-col weight load
                    nc.tensor.matmul(
                        psums[0:1, :], ones[:, 0:1], pe[:, 0:512],
                        start=(jp == 0), stop=False, skip_group_check=True,
                    )
                    nc.tensor.matmul(
                        psums[0:1, :], ones[:, 0:1], pe[:, 512:1024],
                        start=False, stop=last, skip_group_check=True,
                    )
                    pop_op()
                if causal:
                    jb = 4 * c
                    p1 = pe_d1.pop((i, c))
                    p2 = pe_d2.pop((i, c))
                    segs = [
                        (p1[:, 0:512], 0),
                        (p1[:, 512:896], 128),
                        (p2[:, 0:256], 256),
                        (p2[:, 256:384], 384),
                    ]
                    for r, (pes, col) in enumerate(segs):
                        vsl = v16[:, (jb + r) * 128 : (jb + r + 1) * 128]
                        nc.tensor.matmul(
                            pattn[:, col:512], vsl, pes,
                            start=(c == 0 and r == 0), stop=(r == 3),
                            skip_group_check=True,
                        )
                    pop_op()
                    for r, (pes, col) in enumerate(segs):
                        nc.tensor.matmul(
                            psums[0:1, col:512], ones[:, 0:1], pes,
                            start=(c == 0 and r == 0), stop=(r == 3),
                            skip_group_check=True,
                        )
                    pop_op()
                # drain the accumulators: row sums + unnormalized attnT
                sums32 = small.tile([1, 512], F32, tag="sums32")
                nc.vector.tensor_copy(out=sums32[:], in_=psums[0:1, :])
                attn16 = small.tile([128, 512], F16, tag="attn16")
                nc.vector.tensor_copy(out=attn16[:], in_=pattn[:])
                # broadcast row sums -> per-partition columns, written into
                # the (now drained) psums bank's last columns, and take the
                # reciprocal eagerly so it's long ready when the out-proj
                # units pop
                for ls in range(4):
                    nc.tensor.matmul(
                        psums[:, 508 + ls : 509 + ls],
                        sums32[0:1, ls * 128 : (ls + 1) * 128],
                        e1_sb[0:1, :],
                    )
                recip = recs.tile([128, 4], F32, tag="recip")
                nc.vector.reciprocal(out=recip[:], in_=psums[:, 508:512])
                return recip, attn16

            op_queue = []  # per-l-tile out-proj emitters, interleaved into
            # the next chunk's attnV stream so the drain engines keep pace

            def outproj_units(i, c, recip, attn16):
                def ltile(ls):
                    def emit():
                        lt = 4 * c + ls
                        a_sl = attn16[:, ls * 128 : (ls + 1) * 128]
                        r_sl = recip[:, ls : ls + 1]
                        ost = outsp.tile([128, D], F16, tag="ost")
                        for dp in range(2):
                            # cycle po through sp(x2) + pproj(x1) slots: a
                            # 3-deep ring so PE doesn't wait on the drains
                            k = 2 * ls + dp
                            if k % 3 == 2:
                                po = pbig.tile([128, 1024], F32, tag="pproj")
                            else:
                                po = pbig.tile([128, 1024], F32, tag="sp", bufs=2)
                            nc.tensor.matmul(
                                po[:, 0:512],
                                a_sl,
                                wo_sb[:, i, dp * 1024 : dp * 1024 + 512],
                            )
                            nc.tensor.matmul(
                                po[:, 512:1024],
                                a_sl,
                                wo_sb[:, i, dp * 1024 + 512 : dp * 1024 + 1024],
                            )
                            osl = slice(dp * 1024, (dp + 1) * 1024)
                            # one full-width scaled drain per po tile,
                            # alternating engines: a single 1024-wide op
                            # amortizes the per-instruction overhead
                            if dp == 0:
                                nc.vector.tensor_scalar_mul(
                                    out=ost[:, osl], in0=po[:], scalar1=r_sl
                                )
                            else:
                                nc.scalar.activation(
                                    out=ost[:, osl], in_=po[:],
                                    func=CPY, scale=r_sl,
                                )
                        nc.scalar.dma_start(out=out[i, lt], in_=ost[:])

                    return emit

                return [ltile(ls) for ls in range(4)]

            def pop_op():
                if op_queue:
                    op_queue.pop(0)()

            # head-1 chunks rotated so the serial tail (last attn_phase +
            # its out-proj) lands on the cheapest chunk (c=0, diag only)
            order = [(0, c) for c in range(NCH)] + [
                (1, 1), (1, 2), (1, 3), (1, 0)
            ]
            for i, c in order:
                drain_until(i, c)
                recip, attn16 = attn_phase(i, c)
                op_queue.extend(outproj_units(i, c, recip, attn16))
            while op_queue:
                pop_op()
    nc.compile()
    return nc


def _get_program(causal: bool) -> bass.Bass:
    if causal not in _programs:
        _programs[causal] = _build_program(causal)
    return _programs[causal]


def _rope_tables(position_ids: np.ndarray):
    pos = position_ids.reshape(-1).astype(np.float32)  # (L,)
    inv_freq = (
        1.0 / (THETA ** (np.arange(0, HD, 2, dtype=np.float32) / HD))
    ).astype(np.float32)
    freqs = pos[:, None] * inv_freq[None, :]  # (L, HD/2)
    emb = np.concatenate([freqs, freqs], axis=1)  # (L, HD)
    cos = np.cos(emb).T.astype(np.float32).copy()  # (HD, L)
    sin = np.sin(emb).T.astype(np.float32).copy()
    return cos, sin


def _xt_tiles(x):  # (L, D) fp32 -> (NDT, 128, L) fp16 transposed tiles
    return np.ascontiguousarray(x.T.astype(np.float16).reshape(NDT, 128, L))


def kernel(
    q_hidden, k_hidden, v_hidden, wq, wk, wv, wo, attention_mask, position_ids
):
    global last_exec_time_ns, last_mean_exec_time_ns
    q_hidden = np.asarray(q_hidden)
    k_hidden = np.asarray(k_hidden)
    v_hidden = np.asarray(v_hidden)
    wq = np.asarray(wq, dtype=np.float32)
    wk = np.asarray(wk, dtype=np.float32)
    wv = np.asarray(wv, dtype=np.float32)
    wo = np.asarray(wo, dtype=np.float32)
    attention_mask = np.asarray(attention_mask, dtype=np.float32)
    position_ids = np.asarray(position_ids)

    mask2d = attention_mask.reshape(L, L)
    causal_ref = np.where(
        np.tril(np.ones((L, L), dtype=bool)), np.float32(0.0), np.float32(-1e9)
    )
    causal = bool(np.array_equal(mask2d, causal_ref))

    cos, sin = _rope_tables(position_ids)
    scale = np.float32(1.0 / np.sqrt(HD))
    cosq_h = (cos * scale).astype(np.float16)
    sinq_h = (sin * scale).astype(np.float16)
    cosk_h = cos.astype(np.float16)
    sink_h = sin.astype(np.float16)

    jj = np.arange(128, dtype=np.int32)[:, None]
    ll = np.arange(128, dtype=np.int32)[None, :]
    trimask_h = np.where(jj <= ll, np.float32(0.0), np.float32(-1e9)).astype(
        np.float32
    )

    misc_h = np.zeros((128, 256), dtype=np.float16)
    # rotate-half: rh = P @ q with P[i, i+64] = -1 (i<64), P[i, i-64] = +1;
    # stored as lhsT = P^T
    for a in range(64):
        misc_h[a, a + 64] = np.float16(1.0)  # P^T[a, a+64] = P[a+64, a] = +1
        misc_h[a + 64, a] = np.float16(-1.0)  # P^T[a+64, a] = P[a, a+64] = -1
    misc_h[:, 128:256] = np.float16(1.0)  # ones block
    e1_h = np.zeros((128, 1), dtype=np.float32)
    e1_h[0, 0] = 1.0

    wq_r = wq.reshape(H, HD, D)
    wk_r = wk.reshape(HKV, HD, D)
    wv_r = wv.reshape(HKV, HD, D)
    wo_r = wo.reshape(D, H, HD)

    if not causal:
        # (128, NLT//2, NCH, 1024): pair jp holds j-tiles 2jp | 2jp+1
        mt = mask2d.T.reshape(NLT, 128, NCH, 512)
        maskg_h = np.ascontiguousarray(
            np.concatenate([mt[0::2], mt[1::2]], axis=3).transpose(1, 0, 2, 3)
        ).astype(np.float32)

    in_maps = []
    for core in range(NC):
        heads = [HPC * core + i for i in range(HPC)]
        g = heads[0] // (H // HKV)
        # weights: lhsT layout W^T tiles, partition-major
        wq_t = np.stack(
            [
                wq_r[n].T.astype(np.float16).reshape(NDT, 128, HD)
                for n in heads
            ],
            axis=0,
        )  # (HPC, NDT, 128p, 128m)
        wq_t = np.ascontiguousarray(wq_t.transpose(2, 0, 1, 3))  # (128, HPC, NDT, 128)
        wk_t = wk_r[g].T.astype(np.float16).reshape(NDT, 128, HD)
        wk_t = np.ascontiguousarray(wk_t.transpose(1, 0, 2))  # (128, NDT, 128)
        wv_t = wv_r[g].T.astype(np.float16).reshape(NDT, 128, HD)
        wv_t = np.ascontiguousarray(wv_t.transpose(1, 0, 2))
        wo_t = np.stack(
            [wo_r[:, n, :].T.astype(np.float16) for n in heads], axis=0
        )  # (HPC, 128, D)
        wo_t = np.ascontiguousarray(wo_t.transpose(1, 0, 2))  # (128, HPC, D)

        m = {
            "xq": np.stack([_xt_tiles(q_hidden[n, 0]) for n in heads], axis=0),
            "xk": _xt_tiles(k_hidden[g, 0]),
            "xv": _xt_tiles(v_hidden[g, 0]),
            "wq": wq_t,
            "wk": wk_t,
            "wv": wv_t,
            "wo": wo_t,
            "cosq": cosq_h,
            "sinq": sinq_h,
            "cosk": cosk_h,
            "sink": sink_h,
            "misc": misc_h,
            "e1": e1_h,
        }
        if causal:
            m["trimask"] = trimask_h
        else:
            m["maskg"] = maskg_h
        in_maps.append(m)

    nc = _get_program(causal)
    trace_env = os.environ.get("KERNEL_TRACE", "0")
    kwargs = {}
    if trace_env != "0":
        kwargs["trace"] = True
        if trace_env == "8":
            kwargs["trace_cores"] = list(range(NC))
    res = run_bass_kernel_spmd(nc, in_maps, core_ids=list(range(NC)), **kwargs)
    last_exec_time_ns = res.exec_time_ns
    last_mean_exec_time_ns = res.mean_exec_time_ns
    globals()["last_results"] = res.results
    globals()["last_in_maps"] = in_maps
    globals()["last_res"] = res

    out = np.empty((H, 1, L, D), dtype=np.float32)
    for core in range(NC):
        o = res.results[core]["out"]  # (HPC, NLT, 128, D) fp16
        for i in range(HPC):
            out[HPC * core + i, 0] = o[i].reshape(L, D).astype(np.float32)
    return out
